# revision 4
# baseline (speedup 1.0000x reference)
"""HDC binary attention v5 — fp8 DoubleRow kernel for 8 trn2 NeuronCores.

See kernel_v2/v3 docstrings. v5 scheduling changes vs v4:
  - DMA queue discipline: a DMA holds its issuing engine's SEQ while
    waiting on semaphores, so input DMAs (no deps) all go on SP in a
    hand-tuned order; out DMAs go on Pool (SWDGE path, also relieving the
    shared HWDGE); slot 7's final half goes on SP (idle by then).
  - skt chunked (2,2,4,4,4 tiles) so QK starts after ~2 small DMAs; vq3
    split in half so the last input transfer is small.
  - Final copies split ACT/DVE so the tail runs them in parallel.
"""

import numpy as np

B, T, D = 4, 2048, 1024
NCORES = 8
ST = 16
DT = 8
NS = 8

QTILES = {0: [0, 2, 4, 6, 9, 11, 13, 15], 1: [1, 3, 5, 7, 8, 10, 12, 14]}

_CACHE = {}


def build_nc():
    import concourse.bacc as bacc
    import concourse.mybir as mybir
    import concourse.tile as tile

    fp32 = mybir.dt.float32
    bf16 = mybir.dt.bfloat16
    fp8 = mybir.dt.float8e4
    AF = mybir.ActivationFunctionType
    DR = mybir.MatmulPerfMode.DoubleRow

    nc = bacc.Bacc("TRN2", target_bir_lowering=False, debug=False)

    SKT_CHUNKS = [(0, 1), (1, 1), (2, 2), (4, 4), (8, 4), (12, 4)]
    skt_d = [nc.dram_tensor(f"skt{ci}", [128, DT, 128 * n], fp8,
                            kind="ExternalInput").ap()
             for ci, (base, n) in enumerate(SKT_CHUNKS)]
    sqt_d = nc.dram_tensor("sqt", [4, 128, DT, 256], fp8, kind="ExternalInput").ap()
    vq_d = nc.dram_tensor("vq", [4, 128, 4, 1024], fp8, kind="ExternalInput").ap()
    mask_d = nc.dram_tensor("mask", [128, ST, 128], fp8, kind="ExternalInput").ap()
    out_d = nc.dram_tensor("out", [NS, 128, 1024], bf16, kind="ExternalOutput").ap()

    with tile.TileContext(nc) as tc:
        with (
            tc.tile_pool(name="const", bufs=1) as constp,
            tc.tile_pool(name="ps", bufs=2, space="PSUM") as psp,
            tc.tile_pool(name="po", bufs=2, space="PSUM") as pop,
            tc.tile_pool(name="outb", bufs=4) as outp,
        ):
            CHUNKS = SKT_CHUNKS
            CH = {}
            for ci, (base, n) in enumerate(CHUNKS):
                for st in range(base, base + n):
                    CH[st] = (ci, (st - base) * 128)
            sktt = [constp.tile([128, DT, 128 * n], fp8, tag=f"skt{ci}",
                                name=f"skt{ci}")
                    for ci, (base, n) in enumerate(CHUNKS)]
            sqtt = [constp.tile([128, DT, 256], fp8, tag=f"sqt{c}", name=f"sqt{c}")
                    for c in range(4)]
            vqt = [constp.tile([128, 4, 1024], fp8, tag=f"vq{g}", name=f"vq{g}")
                   for g in range(4)]
            msk = constp.tile([128, ST, 128], fp8, tag="msk")
            att = constp.tile([128, ST, 1024], fp8, tag="att")

            def d_skt(ci):
                nc.sync.dma_start(sktt[ci][:], skt_d[ci])

            nc.sync.dma_start(sqtt[0][:], sqt_d[0])
            d_skt(0)
            d_skt(1)
            nc.sync.dma_start(sqtt[1][:], sqt_d[1])
            nc.sync.dma_start(msk[:], mask_d)
            nc.sync.dma_start(sqtt[2][:], sqt_d[2])
            nc.sync.dma_start(sqtt[3][:], sqt_d[3])
            d_skt(2)
            nc.sync.dma_start(vqt[0][:], vq_d[0])
            d_skt(3)
            nc.sync.dma_start(vqt[1][:], vq_d[1])
            d_skt(4)
            nc.sync.dma_start(vqt[2][:], vq_d[2])
            nc.sync.dma_start(vqt[3][:, 0:2, :], vq_d[3][:, 0:2, :])
            d_skt(5)
            nc.sync.dma_start(vqt[3][:, 2:4, :], vq_d[3][:, 2:4, :])

            def qk(st):
                i0 = st // 2
                a0 = 128 * i0
                ps = psp.tile([128, 1024], fp32, tag="ps", name=f"ps{st}")
                ci, koff = CH[st]
                c0 = a0
                while c0 < 1024:
                    cw = 128 if (c0 % 256) else min(256, 1024 - c0)
                    cc, co = c0 // 256, c0 % 256
                    for m in range(4):
                        nc.tensor.matmul(
                            ps[:, c0:c0 + cw],
                            sktt[ci][:, 2 * m:2 * m + 2, koff:koff + 128],
                            sqtt[cc][:, 2 * m:2 * m + 2, co:co + cw],
                            start=(m == 0), stop=(m == 3),
                            perf_mode=DR,
                        )
                    c0 += cw
                nc.scalar.activation(att[:, st, a0:1024], ps[:, a0:1024],
                                     AF.Tanh, scale=1.0 / 16.0)
                nc.vector.tensor_mul(att[:, st, a0:a0 + 128],
                                     att[:, st, a0:a0 + 128],
                                     msk[:, st, :])

            def av_chain(i, po, half, cd):
                for pp in range(i + 1):
                    nc.tensor.matmul(
                        po[:, cd * 256:(cd + 1) * 256],
                        att[:, 2 * pp:2 * pp + 2, i * 128:(i + 1) * 128],
                        vqt[pp // 2][:, 2 * (pp % 2):2 * (pp % 2) + 2,
                                     half * 512 + cd * 256:
                                     half * 512 + (cd + 1) * 256],
                        start=(pp == 0), stop=(pp == i),
                        perf_mode=DR,
                    )

            def av(i):
                ob = outp.tile([128, 1024], bf16, tag="ob", name=f"ob{i}")
                for half in range(2):
                    po = pop.tile([128, 512], fp32, tag=f"po{half}",
                                  name=f"po{i}_{half}")
                    av_chain(i, po, half, 0)
                    av_chain(i, po, half, 1)
                    dst = ob[:, half * 512:(half + 1) * 512]
                    if half == 0 and i == 7:
                        nc.scalar.activation(dst, po[:], AF.Copy)
                    else:
                        nc.vector.tensor_copy(dst, po[:])
                    if half == 0:
                        nc.gpsimd.dma_start(out_d[i, :, 0:512], ob[:, 0:512])
                    elif i == 7:
                        nc.scalar.dma_start(out_d[i, :, 512:1024],
                                            ob[:, 512:1024])
                    else:
                        nc.sync.dma_start(out_d[i, :, 512:1024],
                                          ob[:, 512:1024])

            for st in range(ST):
                qk(st)
                if st % 2 == 1:
                    av(st // 2)

    nc.compile()
    return nc


def host_inputs(x, bv_q, bv_k, bv_v):
    import ml_dtypes
    f8 = ml_dtypes.float8_e4m3

    x = np.asarray(x, dtype=np.float32)
    sq = np.sign(np.asarray(bv_q, dtype=np.float32))
    sk = np.sign(np.asarray(bv_k, dtype=np.float32))
    sv = np.sign(np.asarray(bv_v, dtype=np.float32))
    c = (sq * sk).astype(np.float32)

    in_maps = []
    corr = []
    for core in range(NCORES):
        b, parity = core // 2, core % 2
        L = QTILES[parity]
        S = np.sign(x[b]).astype(np.float32)
        V = x[b] * sv

        sktf = (S.reshape(16, 128, DT, 128)           # [st, j, k, p]
                .transpose(0, 3, 2, 1))               # [st, p, k, j]
        skt_chunks = {}
        for ci, (base, n) in enumerate(
                [(0, 1), (1, 1), (2, 2), (4, 4), (8, 4), (12, 4)]):
            # [n, p, k, j] -> [p, k, n*128] with col = 128*u + j
            skt_chunks[f"skt{ci}"] = np.ascontiguousarray(
                sktf[base:base + n].transpose(1, 2, 0, 3)
                .reshape(128, DT, n * 128)).astype(f8)
        CS = S * c
        qrows = np.concatenate(
            [np.arange(128 * l, 128 * l + 128) for l in L])
        sqt = np.ascontiguousarray(
            CS[qrows].reshape(4, 2, 128, DT, 128)
            .transpose(0, 4, 3, 1, 2)
            .reshape(4, 128, DT, 256)).astype(f8)
        vq = np.ascontiguousarray(
            (0.5 * V).reshape(4, 4, 128, D)
            .transpose(0, 2, 1, 3)).astype(f8)

        mask = np.zeros((128, ST, 128), np.float32)
        p_idx = np.arange(128)[:, None]
        j_idx = np.arange(128)[None, :]
        for i in range(NS):
            qrow = 128 * L[i]
            for bnd in range(2):
                st = 2 * i + bnd
                mask[:, st, :] = ((128 * st + p_idx) <= (qrow + j_idx))
        im = {"sqt": sqt, "vq": vq, "mask": mask.astype(f8)}
        im.update(skt_chunks)
        in_maps.append(im)
        if parity == 0:
            corr.append(0.5 * np.cumsum(V.astype(np.float64), axis=0))
    return in_maps, corr


def assemble_output(results, corr):
    out = np.zeros((B, T, D), np.float32)
    for core in range(NCORES):
        b, parity = core // 2, core % 2
        L = QTILES[parity]
        o = np.asarray(results[core]["out"], dtype=np.float32)
        for i in range(NS):
            r0 = 128 * L[i]
            out[b, r0:r0 + 128] = o[i] + corr[b][r0:r0 + 128]
    return out


def kernel(x, bv_q, bv_k, bv_v):
    from concourse.bass_utils import run_bass_kernel_spmd

    if "nc" not in _CACHE:
        _CACHE["nc"] = build_nc()
    nc = _CACHE["nc"]

    in_maps, corr = host_inputs(x, bv_q, bv_k, bv_v)
    res = run_bass_kernel_spmd(nc, in_maps, list(range(NCORES)))
    _CACHE["last_result"] = res
    return assemble_output(res.results, corr)


# revision 5
# speedup vs baseline: 1.0221x; 1.0221x over previous
"""HDC binary attention — fp8 DoubleRow kernel for 8 trn2 NeuronCores.

Problem: B,T,D = 4,2048,1024
    Q = sign(x*sign(bv_q)); K = sign(x*sign(bv_k)); V = x*sign(bv_v)
    scores = (Q@K^T)/sqrt(D), causal; out = sigmoid(4*scores)*mask @ V

Math. sign(x*b) = sign(x)*sign(b), so with S = sign(x) and
c = sign(bv_q)*sign(bv_k):
    raw[t,s] = sum_d c_d S[t,d] S[s,d]     (integer)
    att      = sigmoid(raw/8) = 0.5 + 0.5*tanh(raw/16)
Decompose att = 0.5*causal + att' with att' = 0.5*tanh(raw/16)*causal:
    out = att'@V + 0.5*cumsum(V)[t]
The cumsum correction is independent of the scores, so the HOST computes it
exactly (fp64) and adds it during output assembly. The device only computes
    att''@(0.5*V),  att'' = tanh(raw/16)*mask
with EVERYTHING in fp8 DoubleRow matmuls (0.5 cyc/row, 256-deep
contraction): QK^T on +-1 signs is exact in fp8; the AV product's fp8
error was measured end-to-end at ~1.0e-2 max-rel (tolerance 2e-2).

Sharding: 2 cores per batch. Core parity p owns eight 128-row q-tiles
    L_0 = [0,2,4,6,9,11,13,15], L_1 = [1,3,5,7,8,10,12,14]
(pairing tile t with 15-t balances causal work exactly: slot i attends
canonical s-tiles 0..2i+1). All per-core variation (which q rows, causal
boundary shape) is carried in host-built inputs (sqt, mask), so the device
program is SPMD-uniform. Per core: 160 QK + 144 AV DoubleRow matmuls,
tanh on ACT (fp8 out), boundary mask mul on DVE, PSUM->bf16 copies split
ACT/DVE, out as bf16 (host upcasts).

Scheduling notes (cost-model-driven):
  - A DMA holds its issuing engine's SEQ while waiting on semaphores, so
    input DMAs (no deps) all go on SP in a hand-tuned arrival order; out
    DMAs go per-half on Pool (SWDGE path, relieves the shared HWDGE) and
    SP/ACT; the last transfers are small and on otherwise-idle queues.
  - Every DMA is a whole-tensor contiguous copy (src/dst AP iteration
    orders must match; >=512B runs avoid the half-bandwidth penalty).
  - PSUM allows ONE open accumulation group per 2KB bank: all matmul
    chains open and close sequentially per bank (psq 2x[128,1024] for QK,
    pav 2x2x[128,512] for AV = exactly 8 banks).
Timeline: 29.9us cost-model (108.5us baseline): ~4us startup DMA latency,
~21us DMA-bound middle fully overlapped with 16.7us of PE, ~4.5us tail
(last AV chain -> copy -> descriptor-gen -> transfer -> completion sem).
"""

import numpy as np

B, T, D = 4, 2048, 1024
NCORES = 8
ST = 16
DT = 8
NS = 8

QTILES = {0: [0, 2, 4, 6, 9, 11, 13, 15], 1: [1, 3, 5, 7, 8, 10, 12, 14]}

_CACHE = {}


def build_nc():
    import concourse.bacc as bacc
    import concourse.mybir as mybir
    import concourse.tile as tile

    fp32 = mybir.dt.float32
    bf16 = mybir.dt.bfloat16
    fp8 = mybir.dt.float8e4
    AF = mybir.ActivationFunctionType
    DR = mybir.MatmulPerfMode.DoubleRow

    nc = bacc.Bacc("TRN2", target_bir_lowering=False, debug=False)

    SKT_CHUNKS = [(0, 1), (1, 1), (2, 2), (4, 4), (8, 4), (12, 4)]
    skt_d = [nc.dram_tensor(f"skt{ci}", [128, DT, 128 * n], fp8,
                            kind="ExternalInput").ap()
             for ci, (base, n) in enumerate(SKT_CHUNKS)]
    sqt_d = nc.dram_tensor("sqt", [4, 128, DT, 256], fp8, kind="ExternalInput").ap()
    vq_d = nc.dram_tensor("vq", [4, 128, 4, 1024], fp8, kind="ExternalInput").ap()
    mask_d = nc.dram_tensor("mask", [128, ST, 128], fp8, kind="ExternalInput").ap()
    out_d = nc.dram_tensor("out", [NS, 128, 1024], bf16, kind="ExternalOutput").ap()

    with tile.TileContext(nc) as tc:
        with (
            tc.tile_pool(name="const", bufs=1) as constp,
            tc.tile_pool(name="ps", bufs=2, space="PSUM") as psp,
            tc.tile_pool(name="po", bufs=2, space="PSUM") as pop,
            tc.tile_pool(name="outb", bufs=4) as outp,
        ):
            CHUNKS = SKT_CHUNKS
            CH = {}
            for ci, (base, n) in enumerate(CHUNKS):
                for st in range(base, base + n):
                    CH[st] = (ci, (st - base) * 128)
            sktt = [constp.tile([128, DT, 128 * n], fp8, tag=f"skt{ci}",
                                name=f"skt{ci}")
                    for ci, (base, n) in enumerate(CHUNKS)]
            sqtt = [constp.tile([128, DT, 256], fp8, tag=f"sqt{c}", name=f"sqt{c}")
                    for c in range(4)]
            vqt = [constp.tile([128, 4, 1024], fp8, tag=f"vq{g}", name=f"vq{g}")
                   for g in range(4)]
            msk = constp.tile([128, ST, 128], fp8, tag="msk")
            att = constp.tile([128, ST, 1024], fp8, tag="att")

            def d_skt(ci):
                nc.sync.dma_start(sktt[ci][:], skt_d[ci])

            nc.sync.dma_start(sqtt[0][:], sqt_d[0])
            d_skt(0)
            d_skt(1)
            nc.sync.dma_start(sqtt[1][:], sqt_d[1])
            nc.sync.dma_start(msk[:], mask_d)
            nc.sync.dma_start(sqtt[2][:], sqt_d[2])
            nc.sync.dma_start(sqtt[3][:], sqt_d[3])
            d_skt(2)
            nc.sync.dma_start(vqt[0][:], vq_d[0])
            d_skt(3)
            nc.sync.dma_start(vqt[1][:], vq_d[1])
            d_skt(4)
            nc.sync.dma_start(vqt[2][:], vq_d[2])
            nc.sync.dma_start(vqt[3][:, 0:2, :], vq_d[3][:, 0:2, :])
            d_skt(5)
            nc.sync.dma_start(vqt[3][:, 2:4, :], vq_d[3][:, 2:4, :])

            def qk(st):
                i0 = st // 2
                a0 = 128 * i0
                ps = psp.tile([128, 1024], fp32, tag="ps", name=f"ps{st}")
                ci, koff = CH[st]
                c0 = a0
                while c0 < 1024:
                    cw = 128 if (c0 % 256) else min(256, 1024 - c0)
                    cc, co = c0 // 256, c0 % 256
                    for m in range(4):
                        nc.tensor.matmul(
                            ps[:, c0:c0 + cw],
                            sktt[ci][:, 2 * m:2 * m + 2, koff:koff + 128],
                            sqtt[cc][:, 2 * m:2 * m + 2, co:co + cw],
                            start=(m == 0), stop=(m == 3),
                            perf_mode=DR,
                        )
                    c0 += cw
                nc.scalar.activation(att[:, st, a0:1024], ps[:, a0:1024],
                                     AF.Tanh, scale=1.0 / 16.0)
                nc.vector.tensor_mul(att[:, st, a0:a0 + 128],
                                     att[:, st, a0:a0 + 128],
                                     msk[:, st, :])

            def av_chain(i, po, half, cd):
                for pp in range(i + 1):
                    nc.tensor.matmul(
                        po[:, cd * 256:(cd + 1) * 256],
                        att[:, 2 * pp:2 * pp + 2, i * 128:(i + 1) * 128],
                        vqt[pp // 2][:, 2 * (pp % 2):2 * (pp % 2) + 2,
                                     half * 512 + cd * 256:
                                     half * 512 + (cd + 1) * 256],
                        start=(pp == 0), stop=(pp == i),
                        perf_mode=DR,
                    )

            def av(i):
                ob = outp.tile([128, 1024], bf16, tag="ob", name=f"ob{i}")
                for half in range(2):
                    po = pop.tile([128, 512], fp32, tag=f"po{half}",
                                  name=f"po{i}_{half}")
                    av_chain(i, po, half, 0)
                    av_chain(i, po, half, 1)
                    dst = ob[:, half * 512:(half + 1) * 512]
                    if half == 0 and i == 7:
                        nc.scalar.activation(dst, po[:], AF.Copy)
                    else:
                        nc.vector.tensor_copy(dst, po[:])
                    if half == 0:
                        nc.gpsimd.dma_start(out_d[i, :, 0:512], ob[:, 0:512])
                    elif i == 7:
                        nc.scalar.dma_start(out_d[i, :, 512:1024],
                                            ob[:, 512:1024])
                    else:
                        nc.sync.dma_start(out_d[i, :, 512:1024],
                                          ob[:, 512:1024])

            for st in range(ST):
                qk(st)
                if st % 2 == 1:
                    av(st // 2)

    nc.compile()
    return nc


def host_inputs(x, bv_q, bv_k, bv_v):
    import ml_dtypes
    f8 = ml_dtypes.float8_e4m3

    x = np.asarray(x, dtype=np.float32)
    sq = np.sign(np.asarray(bv_q, dtype=np.float32))
    sk = np.sign(np.asarray(bv_k, dtype=np.float32))
    sv = np.sign(np.asarray(bv_v, dtype=np.float32))
    c = (sq * sk).astype(np.float32)

    in_maps = []
    corr = []
    for core in range(NCORES):
        b, parity = core // 2, core % 2
        L = QTILES[parity]
        S = np.sign(x[b]).astype(np.float32)
        V = x[b] * sv

        sktf = (S.reshape(16, 128, DT, 128)           # [st, j, k, p]
                .transpose(0, 3, 2, 1))               # [st, p, k, j]
        skt_chunks = {}
        for ci, (base, n) in enumerate(
                [(0, 1), (1, 1), (2, 2), (4, 4), (8, 4), (12, 4)]):
            # [n, p, k, j] -> [p, k, n*128] with col = 128*u + j
            skt_chunks[f"skt{ci}"] = np.ascontiguousarray(
                sktf[base:base + n].transpose(1, 2, 0, 3)
                .reshape(128, DT, n * 128)).astype(f8)
        CS = S * c
        qrows = np.concatenate(
            [np.arange(128 * l, 128 * l + 128) for l in L])
        sqt = np.ascontiguousarray(
            CS[qrows].reshape(4, 2, 128, DT, 128)
            .transpose(0, 4, 3, 1, 2)
            .reshape(4, 128, DT, 256)).astype(f8)
        vq = np.ascontiguousarray(
            (0.5 * V).reshape(4, 4, 128, D)
            .transpose(0, 2, 1, 3)).astype(f8)

        mask = np.zeros((128, ST, 128), np.float32)
        p_idx = np.arange(128)[:, None]
        j_idx = np.arange(128)[None, :]
        for i in range(NS):
            qrow = 128 * L[i]
            for bnd in range(2):
                st = 2 * i + bnd
                mask[:, st, :] = ((128 * st + p_idx) <= (qrow + j_idx))
        im = {"sqt": sqt, "vq": vq, "mask": mask.astype(f8)}
        im.update(skt_chunks)
        in_maps.append(im)
        if parity == 0:
            corr.append(0.5 * np.cumsum(V.astype(np.float64), axis=0))
    return in_maps, corr


def assemble_output(results, corr):
    out = np.zeros((B, T, D), np.float32)
    for core in range(NCORES):
        b, parity = core // 2, core % 2
        L = QTILES[parity]
        o = np.asarray(results[core]["out"], dtype=np.float32)
        for i in range(NS):
            r0 = 128 * L[i]
            out[b, r0:r0 + 128] = o[i] + corr[b][r0:r0 + 128]
    return out


def kernel(x, bv_q, bv_k, bv_v):
    from concourse.bass_utils import run_bass_kernel_spmd

    if "nc" not in _CACHE:
        _CACHE["nc"] = build_nc()
    nc = _CACHE["nc"]

    in_maps, corr = host_inputs(x, bv_q, bv_k, bv_v)
    res = run_bass_kernel_spmd(nc, in_maps, list(range(NCORES)))
    _CACHE["last_result"] = res
    return assemble_output(res.results, corr)
